# revision 36
# baseline (speedup 1.0000x reference)
"""Trainium2 Bass kernel for nn_AutoDecoderLayer (dense transformer layer,
feature-dim attention), tensor-parallel over 8 NeuronCores.

Math (per head h):
  Q = emb @ Wq[h].T + bq ; K = emb @ Wk[h].T + bk ; V = hist @ Wv[h].T + bv
  scores = K.T @ Q / sqrt(E)          # [E, E]
  A = softmax(scores, axis=-1)
  Zh = V @ A
  O = sum_h Zh @ Wz[:, hE:(h+1)E].T + bz
  LN1 = layernorm(O) + emb ; FN = LN1 @ WfT + bf ; out = layernorm(FN) + LN1

Sharding: head h -> core h (8 heads, 8 cores). The Gram matrix
G = emb.T @ emb is seq-sharded: each core computes a 512-row partial
Gram and the partials are AllReduced in two 1 MB column chunks (the
emb column-sum rides chunk A as two extra rows). Row-parallel Wz
partials (P) are AllReduced with the row-bias folded in as an extra
row; each core finishes LN/FF on its 512 seq rows; the host
concatenates the 8 row-shards.

Gram trick: since S is contracted inside K.T @ Q,
  scores.T = WqT.T @ G @ WkT + rank-1 bias terms
which avoids materializing Q/K. Computing scores TRANSPOSED makes the
softmax denominator a partition-axis sum (ones-vector matmuls) and
makes exp(scores.T) directly usable as a matmul stationary operand.
V is folded:  O_partial = hist @ P + (bv @ R + bz/8),
  P = Wv.T @ R,  R = A @ Wzh.T.  History arrives pre-transposed.

SBUF: long-lived [128, 1024] arrays share rotating tag groups (w/x/y/z,
8 slots each); Tile's slot-reuse WAR tracking sequences the generations
(weights -> activations -> tail) without extra SBUF. PSUM tags ps0-ps6
rotate for [128, 512] accumulators; ps7 holds the [128, 8] column
accumulators (the G phase briefly uses all of ps0-ps7).
"""

import os

# RDH makes the chunked AllReduces slower than Mesh
os.environ.setdefault("NEURON_RT_DBG_RDH_CC", "0")

import numpy as np

EMB = 1024
HEADS = 8
SEQ = 4096
NCORES = 8
SHARD = SEQ // NCORES  # 512
LN_EPS = 1e-5
NBLK = EMB // 128  # 8 partition blocks per feature dim
NSH = SHARD // 128  # 4 seq blocks per shard
NCH = EMB // 512  # 2 free-dim chunks of 512


def _build(apply_g1b1, apply_g2b2):
    import concourse.bass as bass  # noqa: F401
    import concourse.mybir as mybir
    import concourse.tile as tile
    from concourse import bacc
    from concourse.masks import make_identity

    dt = mybir.dt
    F32 = dt.float32
    F32R = dt.float32r
    BF16 = dt.bfloat16
    AF = mybir.ActivationFunctionType
    ALU = mybir.AluOpType

    nc = bacc.Bacc("TRN2", target_bir_lowering=False, debug=False,
                   num_devices=NCORES)

    # ---- kernel I/O ----
    emb_s = nc.dram_tensor("emb_s", [SHARD, EMB], BF16, kind="ExternalInput")
    emb = nc.dram_tensor("emb", [SEQ, EMB], BF16, kind="ExternalInput")
    histTs = nc.dram_tensor("histTs", [EMB, SHARD], BF16,
                            kind="ExternalInput")
    embres = nc.dram_tensor("embres", [SHARD, EMB], F32, kind="ExternalInput")
    wqT = nc.dram_tensor("wqT", [EMB, EMB], BF16, kind="ExternalInput")
    wkT = nc.dram_tensor("wkT", [EMB, EMB], BF16, kind="ExternalInput")
    wv = nc.dram_tensor("wv", [EMB, EMB], BF16, kind="ExternalInput")
    wzhT = nc.dram_tensor("wzhT", [EMB, EMB], BF16, kind="ExternalInput")
    wfT = nc.dram_tensor("wfT", [EMB, EMB], BF16, kind="ExternalInput")
    bq_d = nc.dram_tensor("bq", [1, EMB], BF16, kind="ExternalInput")
    sbq_d = nc.dram_tensor("sbq", [1, EMB], BF16, kind="ExternalInput")
    bk_d = nc.dram_tensor("bk", [1, EMB], BF16, kind="ExternalInput")
    bz8_d = nc.dram_tensor("bz8", [1, EMB], F32, kind="ExternalInput")
    bf_d = nc.dram_tensor("bf", [1, EMB], F32R, kind="ExternalInput")
    g1_d = nc.dram_tensor("g1", [1, EMB], F32R, kind="ExternalInput")
    b1_d = nc.dram_tensor("b1", [1, EMB], F32R, kind="ExternalInput")
    g2_d = nc.dram_tensor("g2", [1, EMB], F32R, kind="ExternalInput")
    b2_d = nc.dram_tensor("b2", [1, EMB], F32R, kind="ExternalInput")
    bv_d = nc.dram_tensor("bvcol", [128, NBLK], BF16, kind="ExternalInput")
    ones_d = nc.dram_tensor("onesd", [128, 128], F32R, kind="ExternalInput")
    onesbf_d = nc.dram_tensor("onesbf", [1, 128], BF16, kind="ExternalInput")
    out_ext = nc.dram_tensor("out", [SHARD, EMB], BF16,
                             kind="ExternalOutput")

    cc_warm_in = nc.dram_tensor("cc_warm_in", [128, 8], F32)
    cc_warm_out = nc.dram_tensor("cc_warm_out", [128, 8], F32)
    cc_warm_out2 = nc.dram_tensor("cc_warm_out2", [128, 8], F32)
    # G columns [0:768] come from an AllReduce of per-core 512-row Gram
    # partials (chunk A = cols 0:512 + the emb column-sum as rows
    # 1024/1025; chunk B = cols 512:768). Columns [768:1024] are
    # computed locally over the full sequence (as rows, via symmetry)
    # to fill the PE-idle window before the first collective can begin.
    GCW = [512, 256]
    g_bounce = [nc.dram_tensor(f"g_bounce{ch}",
                               [EMB + 2 * (1 - ch), GCW[ch]], BF16)
                for ch in range(NCH)]
    g_totc = [nc.dram_tensor(f"g_tot{ch}", [EMB + 2 * (1 - ch), GCW[ch]],
                             BF16, addr_space="Shared")
              for ch in range(NCH)]
    # P AllReduce by column halves; row 1024 carries the rowaux bias.
    p_bounce = [nc.dram_tensor(f"p_bounce{ch}", [EMB + 1, 512], BF16)
                for ch in range(NCH)]
    p_totc = [nc.dram_tensor(f"p_tot{ch}", [EMB + 1, 512], BF16,
                             addr_space="Shared") for ch in range(NCH)]

    def mm(out, lhsT, rhs, start, stop):
        nc.tensor.matmul(out, lhsT, rhs, start=start, stop=stop)

    with tile.TileContext(nc) as tc:
        sb = tc.alloc_tile_pool(name="sb", bufs=1)
        psum = tc.alloc_tile_pool(name="psum", bufs=1, space="PSUM")

        def big(group, b, nm, width=EMB, dtype=BF16):
            return sb.tile([128, width], dtype, tag=f"{group}{b}",
                           name=f"{nm}{b}")

        psg = [0]

        def ppair(nm):
            # two PSUM accumulators (ch chunks) from the ps0..ps6 rotation
            a = psum.tile([128, 512], F32, tag=f"ps{psg[0] % 7}",
                          name=f"{nm}a")
            b = psum.tile([128, 512], F32, tag=f"ps{(psg[0] + 1) % 7}",
                          name=f"{nm}b")
            psg[0] += 2
            return [a, b]

        def ptile(nm, shape=(128, 512)):
            t = psum.tile(list(shape), F32, tag=f"ps{psg[0] % 7}", name=nm)
            psg[0] += 1
            return t

        def pcol(nm, shape=(128, NBLK)):
            return psum.tile(list(shape), F32, tag="ps7", name=nm)

        # warm up ncfw at instruction zero (input content is irrelevant).
        # Empirically the first collective's mesh program only starts
        # ~11us after the SECOND collective's trigger arrives, so fire
        # two back-to-back dummies: both complete within ~40us and the
        # real G AllReduce then starts immediately.
        nc.gpsimd.collective_compute(
            "AllReduce", mybir.AluOpType.add,
            replica_groups=[list(range(NCORES))],
            ins=[cc_warm_in.ap().opt()],
            outs=[cc_warm_out.ap().opt()],
        )

        # ---- constants ----
        ones_col = sb.tile([128, 1], F32R, tag="ones_col", name="ones_col")
        nc.scalar.dma_start(ones_col[:], ones_d.ap()[0:128, 0:1])
        ones_row = sb.tile([1, 128], F32R, tag="ones_row", name="ones_row")
        nc.scalar.dma_start(ones_row[:], ones_d.ap()[0:1, 0:128])
        onesbf_col = sb.tile([128, 1], BF16, tag="onesbfc", name="onesbfc")
        nc.scalar.dma_start(onesbf_col[:], onesbf_d.ap()[0:1, 0:128])
        ident = sb.tile([128, 128], F32, tag="ident", name="ident")
        make_identity(nc, ident[:])
        eps_sb = sb.tile([128, 1], F32, tag="eps", name="eps")
        nc.gpsimd.memset(eps_sb[:], LN_EPS)

        bv_sb = sb.tile([128, NBLK], BF16, tag="bv", name="bv")
        nc.scalar.dma_start(bv_sb[:], bv_d.ap())
        # [qs; bq; S*bq] and [bk; ks; bk] for the rank-1 score terms
        aux_lhs = sb.tile([3, EMB], BF16, tag="auxl", name="auxl")
        nc.scalar.dma_start(aux_lhs[1:2, :], bq_d.ap())
        nc.scalar.dma_start(aux_lhs[2:3, :], sbq_d.ap())
        aux_rhs = sb.tile([3, EMB], BF16, tag="auxr", name="auxr")
        nc.scalar.dma_start(aux_rhs[0:1, :], bk_d.ap())
        nc.scalar.dma_start(aux_rhs[2:3, :], bk_d.ap())
        bz8_sb = sb.tile([1, EMB], F32, tag="bz8", name="bz8")
        nc.scalar.dma_start(bz8_sb[:], bz8_d.ap())
        bvr_sb = sb.tile([1, EMB], F32, tag="bvr", name="bvr")

        def mmrow(nm, dtype=F32R):
            # rotating partition-0 row slot for matmul-facing row vectors
            return sb.tile([1, EMB], dtype, tag="mmrow", name=nm)

        def load_w(dram, group, nm):
            ts = []
            for b in range(NBLK):
                t = big(group, b, nm)
                # HWDGE queues: alternate ACT/Pool to halve issue latency
                eng = nc.scalar if b % 2 == 0 else nc.gpsimd
                eng.dma_start(t[:], dram.ap()[b * 128:(b + 1) * 128, :])
                ts.append(t)
            return ts

        # ---- Phase 1: partial Gram over this core's 512 seq rows ----
        # Gpart = emb_s.T @ emb_s, AllReduced in two column chunks.
        e_ts = []
        for si in range(NSH):
            e_t = sb.tile([128, EMB], BF16, tag=f"embs{si}", name=f"embs{si}")
            nc.sync.dma_start(e_t[:], emb_s.ap()[si * 128:(si + 1) * 128, :])
            e_ts.append(e_t)

        # emb column-sum (both halves) rides chunk A as rows 1024/1025
        esum_sb = sb.tile([1, EMB], BF16, tag="esum2", name="esum2")
        for half in range(2):
            esp = pcol(f"esp{half}", (1, 512))
            for si in range(NSH):
                mm(esp[:], onesbf_col[:],
                   e_ts[si][:, half * 512:(half + 1) * 512],
                   start=(si == 0), stop=(si == NSH - 1))
            nc.scalar.copy(esum_sb[0:1, half * 512:(half + 1) * 512], esp[:])
            nc.sync.dma_start(g_bounce[0].ap()[EMB + half:EMB + half + 1, :],
                              esum_sb[0:1, half * 512:(half + 1) * 512])

        for ch in range(NCH):
            cs = slice(ch * 512, ch * 512 + GCW[ch])
            gps = [psum.tile([128, 512], F32, tag=f"ps{c}",
                             name=f"gps{ch}_{c}") for c in range(8)]
            for si in range(NSH):
                for c in range(NBLK):
                    mm(gps[c][:, 0:GCW[ch]],
                       e_ts[si][:, c * 128:(c + 1) * 128],
                       e_ts[si][:, cs],
                       start=(si == 0), stop=(si == NSH - 1))
            for c in range(NBLK):
                pstg = sb.tile([128, 512], BF16, tag="pstage", name="gstage",
                               bufs=4)
                nc.vector.tensor_copy(pstg[:, 0:GCW[ch]],
                                      gps[c][:, 0:GCW[ch]])
                eng = nc.sync if c % 2 == 0 else nc.gpsimd
                eng.dma_start(g_bounce[ch].ap()[c * 128:(c + 1) * 128, :],
                              pstg[:, 0:GCW[ch]])
            nc.gpsimd.collective_compute(
                "AllReduce", mybir.AluOpType.add,
                replica_groups=[list(range(NCORES))],
                ins=[g_bounce[ch].ap().opt()],
                outs=[g_totc[ch].ap().opt()],
            )

        wkT_sb = load_w(wkT, "w", "wkT")   # w gen1
        wqT_sb = load_w(wqT, "x", "wqT")   # x gen1
        G_sb = [big("y", b, "G") for b in range(NBLK)]        # y gen1

        # ---- Phase 1c: local G columns [768:1024] over the full seq ----
        # Computed as G rows [768:1024] = emb[:, 768:1024].T @ emb (full
        # 4096-row contraction, PE work that fills the idle window while
        # the first collective waits out its startup barrier), then
        # transposed on-chip into G_sb[:, 768:1024] via symmetry.
        identB = sb.tile([128, 128], BF16, tag="identB", name="identB")
        make_identity(nc, identB[:])
        NSEQ = SEQ // 128
        engs3 = [nc.sync, nc.scalar, nc.gpsimd]
        gr = [[psum.tile([128, 512], F32, tag=f"ps{2 * r + j}",
                         name=f"gr{r}_{j}") for j in range(2)]
              for r in range(2)]
        for si in range(NSEQ):
            e_t = sb.tile([128, EMB], BF16, tag="embf", name="embf", bufs=8)
            engs3[si % 3].dma_start(
                e_t[:], emb.ap()[si * 128:(si + 1) * 128, :])
            for r in range(2):
                for j in range(2):
                    mm(gr[r][j][:],
                       e_t[:, 768 + r * 128:896 + r * 128],
                       e_t[:, j * 512:(j + 1) * 512],
                       start=(si == 0), stop=(si == NSEQ - 1))
        grow = []
        for r in range(2):
            g_t = sb.tile([128, EMB], BF16, tag=f"grow{r}", name=f"grow{r}")
            for j in range(2):
                nc.vector.tensor_copy(g_t[:, j * 512:(j + 1) * 512],
                                      gr[r][j][:])
            grow.append(g_t)
        for c in range(NBLK):
            tps = psum.tile([128, 256], BF16, tag=f"ps{4 + c % 4}",
                            name=f"gt{c}")
            for r in range(2):
                nc.tensor.transpose(tps[:, r * 128:(r + 1) * 128],
                                    grow[r][:, c * 128:(c + 1) * 128],
                                    identB[:])
            nc.vector.tensor_copy(G_sb[c][:, 768:1024], tps[:])

        # ---- Phase 2: land the AllReduced G columns ----
        for c in range(NBLK):
            eng = nc.sync if c % 2 == 0 else nc.gpsimd
            eng.dma_start(G_sb[c][:, 0:512],
                          g_totc[0].ap()[c * 128:(c + 1) * 128, :])
        esum_row = sb.tile([1, EMB], BF16, tag="esumr", name="esumr")
        nc.sync.dma_start(esum_row[0:1, 0:512],
                          g_totc[0].ap()[EMB:EMB + 1, :])
        nc.sync.dma_start(esum_row[0:1, 512:1024],
                          g_totc[0].ap()[EMB + 1:EMB + 2, :])

        # U = G @ WqT (so scoresT = U.T @ WkT; G symmetric). U's c-block
        # needs only G columns c: c = 6,7 are local (pre-collective),
        # c = 0..3 land with chunk A, c = 4,5 with chunk B.
        U_sb = [big("z", b, "U") for b in range(NBLK)]        # z gen1

        def u_block(c):
            pp = [psum.tile([128, 512], F32, tag=f"ps{6 + ch}",
                            name=f"u{c}_{ch}") for ch in range(NCH)]
            for d in range(NBLK):
                for ch in range(NCH):
                    mm(pp[ch][:], G_sb[d][:, c * 128:(c + 1) * 128],
                       wqT_sb[d][:, ch * 512:(ch + 1) * 512],
                       start=(d == 0), stop=(d == NBLK - 1))
            for ch in range(NCH):
                nc.vector.tensor_copy(U_sb[c][:, ch * 512:(ch + 1) * 512],
                                      pp[ch][:])

        # expT takes over wqT's slots (dead after U); wkT stays live for
        # the score matmuls themselves, so its group is reused by R
        expT_sb = [big("x", b, "expT") for b in range(NBLK)]  # x gen2
        inv_sqrt_e = 1.0 / float(np.sqrt(EMB))

        def sc_mm(pp, f, c, first):
            for ch in range(NCH):
                mm(pp[ch][:], U_sb[c][:, f * 128:(f + 1) * 128],
                   wkT_sb[c][:, ch * 512:(ch + 1) * 512],
                   start=first, stop=False)

        def sc_finish(pp, f, cl):
            for c in cl:
                sc_mm(pp, f, c, False)
            for ch in range(NCH):
                mm(pp[ch][:], aux_lhs[0:3, f * 128:(f + 1) * 128],
                   aux_rhs[0:3, ch * 512:(ch + 1) * 512],
                   start=False, stop=True)
                nc.scalar.activation(expT_sb[f][:, ch * 512:(ch + 1) * 512],
                                     pp[ch][:], AF.Exp, scale=inv_sqrt_e)

        # local columns first: U c6/c7, then scoresT partials for f=0,1
        # (psum banks 0-3 held open until chunk B's columns arrive)
        u_block(6)
        u_block(7)
        sc_pp = {}
        for f in range(2):
            sc_pp[f] = [psum.tile([128, 512], F32, tag=f"ps{2 * f + ch}",
                                  name=f"sc{f}_{ch}") for ch in range(NCH)]
            sc_mm(sc_pp[f], f, 6, True)
            sc_mm(sc_pp[f], f, 7, False)

        # chunk A lands: U c0..3
        for c in range(4):
            u_block(c)

        # chunk-B G columns (these DMAs wait on the second AllReduce;
        # issued now so they don't queue behind the qs/ks section)
        for c in range(NBLK):
            eng = nc.sync if c % 2 == 0 else nc.gpsimd
            eng.dma_start(G_sb[c][:, 512:768],
                          g_totc[1].ap()[c * 128:(c + 1) * 128, :])

        # esum -> column layout, then qs/ks (overlaps the chunk-B wait)
        esum_col = sb.tile([128, NBLK], BF16, tag="esum_col",
                           name="esum_col")
        ecp = pcol("ecp")
        for b in range(NBLK):
            mm(ecp[:, b:b + 1], esum_row[0:1, b * 128:(b + 1) * 128],
               onesbf_col[0:1, 0:1], start=True, stop=True)
        nc.scalar.copy(esum_col[:], ecp[:])
        ksr = mmrow("ksr", dtype=BF16)
        for ch in range(NCH):
            qp = psum.tile([1, 512], F32, tag="ps4", name=f"qsp{ch}")
            kp = psum.tile([1, 512], F32, tag="ps5", name=f"ksp{ch}")
            for b in range(NBLK):
                mm(qp[:], esum_col[:, b:b + 1],
                   wqT_sb[b][:, ch * 512:(ch + 1) * 512],
                   start=(b == 0), stop=(b == NBLK - 1))
                mm(kp[:], esum_col[:, b:b + 1],
                   wkT_sb[b][:, ch * 512:(ch + 1) * 512],
                   start=(b == 0), stop=(b == NBLK - 1))
            nc.vector.tensor_copy(aux_lhs[0:1, ch * 512:(ch + 1) * 512],
                                  qp[:])
            nc.vector.tensor_copy(ksr[0:1, ch * 512:(ch + 1) * 512], kp[:])
        # partition-shift ks into aux_rhs row 1 (DMA moves across partitions)
        nc.scalar.dma_start(aux_rhs[1:2, :], ksr[:])

        # chunk B has landed: U c4/c5, then finish all the scores
        u_block(4)
        u_block(5)
        for f in range(2):
            sc_finish(sc_pp[f], f, [0, 1, 2, 3, 4, 5])
        for f in range(2, NBLK):
            pp = [psum.tile([128, 512], F32, tag=f"ps{2 * (f % 3) + ch}",
                            name=f"sc{f}_{ch}") for ch in range(NCH)]
            sc_mm(pp, f, 0, True)
            sc_finish(pp, f, [1, 2, 3, 4, 5, 6, 7])

        # ---- Phase 4: softmax denominators (ones.T @ expT col-sums) ----
        den_ps = [psum.tile([1, 512], F32, tag=f"ps{6 + ch}",
                            name=f"den{ch}") for ch in range(NCH)]
        for ch in range(NCH):
            for f in range(NBLK):
                mm(den_ps[ch][:], onesbf_col[:],
                   expT_sb[f][:, ch * 512:(ch + 1) * 512],
                   start=(f == 0), stop=(f == NBLK - 1))
        den_row = sb.tile([1, EMB], F32, tag="denr", name="denr")
        for ch in range(NCH):
            nc.vector.tensor_copy(den_row[0:1, ch * 512:(ch + 1) * 512],
                                  den_ps[ch][:])
        dcp = pcol("dcp")
        for b in range(NBLK):
            mm(dcp[:, b:b + 1], den_row[0:1, b * 128:(b + 1) * 128],
               ones_row[0:1, 0:1].bitcast(F32), start=True, stop=True)
        sum_col = sb.tile([128, NBLK], F32, tag="sum_col", name="sum_col")
        nc.scalar.copy(sum_col[:], dcp[:])
        recip = sb.tile([128, NBLK], F32, tag="recip", name="recip")
        nc.vector.reciprocal(recip[:], sum_col[:])

        # ---- Phase 5+6: per o-half: R -> rowaux -> P -> AllReduce ----
        wzhT_sb = load_w(wzhT, "y", "wzhT")                   # y gen2
        wv_sb = load_w(wv, "z", "wv")                         # z gen2
        R_sb = [big("w", b, "R") for b in range(NBLK)]        # w gen2
        rowaux = sb.tile([1, EMB], BF16, tag="rowaux", name="rowaux")
        for ch in range(NCH):
            cs = slice(ch * 512, (ch + 1) * 512)
            for e in range(NBLK):
                ps = ptile("rps")
                for f in range(NBLK):
                    mm(ps[:], expT_sb[f][:, e * 128:(e + 1) * 128],
                       wzhT_sb[f][:, cs],
                       start=(f == 0), stop=(f == NBLK - 1))
                nc.scalar.mul(R_sb[e][:, cs], ps[:], recip[:, e:e + 1])
            ps = ptile("bvrp", (1, 512))
            for e in range(NBLK):
                mm(ps[:], bv_sb[:, e:e + 1], R_sb[e][:, cs],
                   start=(e == 0), stop=(e == NBLK - 1))
            nc.vector.tensor_copy(bvr_sb[0:1, cs], ps[:])
            nc.vector.tensor_add(rowaux[0:1, cs], bvr_sb[0:1, cs],
                                 bz8_sb[0:1, cs])
            nc.sync.dma_start(p_bounce[ch].ap()[EMB:EMB + 1, :],
                              rowaux[0:1, cs])
            for c in range(NBLK):
                ps = ptile("pps")
                for e in range(NBLK):
                    mm(ps[:], wv_sb[e][:, c * 128:(c + 1) * 128],
                       R_sb[e][:, cs],
                       start=(e == 0), stop=(e == NBLK - 1))
                # P is only ever read after the AllReduce (as pt_sb), so
                # stage the eviction straight out to the bounce buffer
                pstg = sb.tile([128, 512], BF16, tag="pstage", name="pstage",
                               bufs=4)
                nc.vector.tensor_copy(pstg[:], ps[:])
                nc.sync.dma_start(
                    p_bounce[ch].ap()[c * 128:(c + 1) * 128, :], pstg[:])
            nc.gpsimd.collective_compute(
                "AllReduce", mybir.AluOpType.add,
                replica_groups=[list(range(NCORES))],
                ins=[p_bounce[ch].ap().opt()],
                outs=[p_totc[ch].ap().opt()],
            )

        # ---- Phase 7: load P_tot + my histT shard; O rows are local now ----
        wfT_sb = load_w(wfT, "y", "wfT")                      # y gen3
        onesbf = sb.tile([1, 128], BF16, tag="onesbf", name="onesbf")
        nc.sync.dma_start(onesbf[:], onesbf_d.ap())
        pt_sb = [big("w", b, "ptot", dtype=BF16) for b in range(NBLK)]
        rowt = sb.tile([1, EMB], BF16, tag="rowaux", name="rowt")
        for ch in range(NCH):
            cs = slice(ch * 512, (ch + 1) * 512)
            for c in range(NBLK):
                eng = nc.sync if c % 2 == 0 else nc.scalar
                eng.dma_start(pt_sb[c][:, cs],
                              p_totc[ch].ap()[c * 128:(c + 1) * 128, :])
            nc.sync.dma_start(rowt[0:1, cs], p_totc[ch].ap()[EMB:EMB + 1, :])
        hs_sb = [big("x", b, "histTs", width=SHARD, dtype=BF16)
                 for b in range(NBLK)]                        # x gen3
        for c in range(NBLK):
            nc.scalar.dma_start(hs_sb[c][:],
                                histTs.ap()[c * 128:(c + 1) * 128, :])

        # ---- Phase 8: tail LN1 -> FF -> LN2, pipelined per RS chunk ----
        def tailrow(nm):
            # "bvr"-tag slot generations: bvr_sb is dead after rowaux
            return sb.tile([1, EMB], F32R, tag="bvr", name=nm)

        def bcast_row(dram, slot, nm):
            src_row = tailrow(f"{nm}row")
            nc.sync.dma_start(src_row[:], dram.ap())
            t = big("z", slot, nm, dtype=F32)
            for ch in range(NCH):
                ps = ptile(f"{nm}ps")
                mm(ps[:], ones_row[:],
                   src_row[0:1, ch * 512:(ch + 1) * 512],
                   start=True, stop=True)
                nc.vector.tensor_copy(t[:, ch * 512:(ch + 1) * 512], ps[:])
            return t

        g1_bc = b1_bc = g2_bc = b2_bc = None
        if apply_g1b1:
            g1_bc = bcast_row(g1_d, 4, "g1bc")
            b1_bc = bcast_row(b1_d, 5, "b1bc")
        if apply_g2b2:
            g2_bc = bcast_row(g2_d, 6, "g2bc")
            b2_bc = bcast_row(b2_d, 7, "b2bc")

        def layer_norm(x_sb, res_sb, out_sb, g_bc, b_bc):
            # bn_stats gives mean+var in one DVE pass (vs sum+square)
            stats = sb.tile([128, 12], F32, tag="ln_st6", name="ln_st6",
                            bufs=4)
            for j in range(2):
                nc.vector.bn_stats(stats[:, j * 6:(j + 1) * 6],
                                   x_sb[:, j * 512:(j + 1) * 512])
            aggr = sb.tile([128, 2], F32, tag="ln_ag", name="ln_ag", bufs=4)
            nc.vector.bn_aggr(aggr[:],
                              stats[:].rearrange("p (a b) -> p a b", a=2))
            std = sb.tile([128, 1], F32, tag="ln_std", name="ln_std", bufs=4)
            nc.scalar.activation(std[:], aggr[:, 1:2], AF.Sqrt,
                                 bias=eps_sb[:])
            rstd = sb.tile([128, 1], F32, tag="ln_rstd", name="ln_rstd",
                           bufs=4)
            nc.vector.reciprocal(rstd[:], std[:])
            t = sb.tile([128, EMB], F32, tag="lnc", name="ln_t", bufs=3)
            nc.vector.tensor_scalar(t[:], x_sb[:], aggr[:, 0:1], rstd[:],
                                    op0=ALU.subtract, op1=ALU.mult)
            if g_bc is None:
                nc.vector.tensor_add(out_sb[:], t[:], res_sb[:])
            else:
                t2 = sb.tile([128, EMB], F32, tag="lnt", name="ln_t2",
                             bufs=2)
                nc.vector.tensor_mul(t2[:], t[:], g_bc[:])
                nc.vector.tensor_add(out_sb[:], t2[:], b_bc[:])
                nc.vector.tensor_add(out_sb[:], out_sb[:], res_sb[:])

        bf_row = tailrow("bf_row")
        nc.sync.dma_start(bf_row[:], bf_d.ap())

        o_tiles = [sb.tile([128, EMB], BF16, tag="o_rows",
                           name=f"o_rows{t}", bufs=4) for t in range(4)]

        def tail_O_half(t, ch):
            # O rows, one column half; all tiles' ch0 halves are emitted
            # before any ch1 half so they hide under the second AllReduce
            cs = slice(ch * 512, (ch + 1) * 512)
            ps = psum.tile([128, 512], F32, tag=f"ps{t * 2 + ch}",
                           name=f"otps{t}{ch}")
            for c in range(NBLK):
                mm(ps[:], hs_sb[c][:, t * 128:(t + 1) * 128],
                   pt_sb[c][:, cs], start=(c == 0), stop=False)
            mm(ps[:], onesbf[:], rowt[0:1, cs], start=False, stop=True)
            nc.vector.tensor_copy(o_tiles[t][:, cs], ps[:])

        ln1_tiles = []

        def tail_ln1(t):
            o_t = o_tiles[t]
            r_t = sb.tile([128, EMB], F32, tag="res_rows", name="res_rows",
                          bufs=4)
            nc.sync.dma_start(r_t[:], embres.ap()[t * 128:(t + 1) * 128, :])
            l1 = big("z", t, "ln1", dtype=F32)                # z gen3 (0-3)
            layer_norm(o_t, r_t, l1, g1_bc, b1_bc)
            ln1_tiles.append(l1)

        def tail_rest(t):
            l1 = ln1_tiles[t]
            l1T = [sb.tile([128, 128], BF16, tag=f"l1T{c}",
                           name=f"l1T{t}_{c}") for c in range(NBLK)]
            for c in range(NBLK):
                ps = ptile(f"trp{t}{c}", (128, 128))
                nc.tensor.transpose(ps[:], l1[:, c * 128:(c + 1) * 128],
                                    ident[:])
                nc.vector.tensor_copy(l1T[c][:], ps[:])
            fn = sb.tile([128, EMB], F32, tag="fn", name="fn", bufs=2)
            pp = ppair("fn")
            for c in range(NBLK):
                for ch in range(NCH):
                    mm(pp[ch][:], l1T[c][:],
                       wfT_sb[c][:, ch * 512:(ch + 1) * 512],
                       start=(c == 0), stop=False)
            for ch in range(NCH):
                mm(pp[ch][:], ones_row[:],
                   bf_row[0:1, ch * 512:(ch + 1) * 512],
                   start=False, stop=True)
                nc.vector.tensor_copy(fn[:, ch * 512:(ch + 1) * 512],
                                      pp[ch][:])
            o2 = sb.tile([128, EMB], BF16, tag="out_rows", name="out_rows",
                         bufs=2)
            layer_norm(fn, l1, o2, g2_bc, b2_bc)
            nc.sync.dma_start(out_ext.ap()[t * 128:(t + 1) * 128, :], o2[:])

        for t in range(4):
            tail_O_half(t, 0)
        for t in range(4):
            tail_O_half(t, 1)
        for t in range(4):
            tail_ln1(t)
        for t in range(4):
            tail_rest(t)

        psum.release()
        sb.release()

    nc.compile()
    return nc


_CACHE = {}


def _get_nc(apply_g1b1, apply_g2b2):
    key = (apply_g1b1, apply_g2b2)
    if key not in _CACHE:
        _CACHE[key] = _build(apply_g1b1, apply_g2b2)
    return _CACHE[key]


def _shard_inputs(history, embdding, Wq_w, Wq_b, Wk_w, Wk_b, Wv_w, Wv_b,
                  Wz_w, Wz_b, ln1_g, ln1_b, Wf_w, Wf_b, ln2_g, ln2_b):
    f32 = np.float32
    import ml_dtypes
    bf16 = ml_dtypes.bfloat16
    emb = np.ascontiguousarray(embdding, dtype=f32)
    emb_bf = np.ascontiguousarray(emb.astype(bf16))
    histT = np.ascontiguousarray(
        np.asarray(history, dtype=f32).T.astype(bf16))
    onesbf = np.ones((1, 128), dtype=bf16)
    wfT = np.ascontiguousarray(np.asarray(Wf_w, dtype=f32).T.astype(bf16))
    ones = np.ones((128, 128), dtype=f32)
    bz8 = (np.asarray(Wz_b, dtype=f32) / NCORES).reshape(1, EMB)
    bf = np.asarray(Wf_b, dtype=f32).reshape(1, EMB)
    g1 = np.asarray(ln1_g, dtype=f32).reshape(1, EMB)
    b1 = np.asarray(ln1_b, dtype=f32).reshape(1, EMB)
    g2 = np.asarray(ln2_g, dtype=f32).reshape(1, EMB)
    b2 = np.asarray(ln2_b, dtype=f32).reshape(1, EMB)
    in_maps = []
    for h in range(NCORES):
        bq = np.asarray(Wq_b[h], dtype=f32).reshape(1, EMB)
        m = {
            "emb_s": np.ascontiguousarray(
                emb_bf[h * SHARD:(h + 1) * SHARD, :]),
            "emb": emb_bf,
            "histTs": np.ascontiguousarray(
                histT[:, h * SHARD:(h + 1) * SHARD]),
            "onesbf": onesbf,
            "embres": np.ascontiguousarray(emb[h * SHARD:(h + 1) * SHARD, :]),
            "wqT": np.ascontiguousarray(
                np.asarray(Wq_w[h], dtype=f32).T.astype(bf16)),
            "wkT": np.ascontiguousarray(
                np.asarray(Wk_w[h], dtype=f32).T.astype(bf16)),
            "wv": np.ascontiguousarray(
                np.asarray(Wv_w[h], dtype=f32).astype(bf16)),
            "wzhT": np.ascontiguousarray(np.asarray(
                Wz_w[:, h * EMB:(h + 1) * EMB], dtype=f32).T.astype(bf16)),
            "wfT": wfT,
            "bq": bq.astype(bf16),
            "sbq": (bq * float(SEQ)).astype(bf16),
            "bk": np.asarray(Wk_b[h], dtype=f32).reshape(1, EMB).astype(bf16),
            "bz8": bz8, "bf": bf,
            "g1": g1, "b1": b1, "g2": g2, "b2": b2,
            "bvcol": np.ascontiguousarray(np.asarray(
                Wv_b[h], dtype=f32).reshape(NBLK, 128).T.astype(bf16)),
            "onesd": ones,
        }
        in_maps.append(m)
    return in_maps


def kernel(history, embdding, Wq_w, Wq_b, Wk_w, Wk_b, Wv_w, Wv_b,
           Wz_w, Wz_b, ln1_g, ln1_b, Wf_w, Wf_b, ln2_g, ln2_b,
           trace=False):
    from concourse.bass_utils import run_bass_kernel_spmd

    apply_g1b1 = not (np.allclose(ln1_g, 1.0) and np.allclose(ln1_b, 0.0))
    apply_g2b2 = not (np.allclose(ln2_g, 1.0) and np.allclose(ln2_b, 0.0))
    nc = _get_nc(apply_g1b1, apply_g2b2)
    in_maps = _shard_inputs(history, embdding, Wq_w, Wq_b, Wk_w, Wk_b,
                            Wv_w, Wv_b, Wz_w, Wz_b, ln1_g, ln1_b,
                            Wf_w, Wf_b, ln2_g, ln2_b)
    res = run_bass_kernel_spmd(nc, in_maps, core_ids=list(range(NCORES)),
                               trace=trace)
    out = np.concatenate(
        [np.asarray(res.results[i]["out"]).astype(np.float32)
         for i in range(NCORES)], axis=0)
    if trace:
        return out, res
    return out
